# revision 32
# baseline (speedup 1.0000x reference)
"""Trainium2 Bass kernel for CustomMultiHeadAttention (B=2, L=2048, D=512, H=8).

Sharding: 8 cores = 2 batches x 4 head-pairs. Each core computes, for its
batch b and its 2 heads, the partial output (O_h @ Wo_h summed over its
heads), transposed: poutT [512, 2048]. Host sums the 4 partials per batch,
transposes, and adds bo.

Device-side math per core (all masking folded into matmul contractions):
  Qh = (q[b]*qm) @ WqT_cols + qm*bq_cols          (masked q rows -> exactly 0)
  Kh = k[b] @ WkT_cols/8 + bk_cols/8
  E[k,q] = Kh.Qh + (kb[k]-c)*qm[q] + c            via 2 extra contraction rows
           (kb = -100 for masked keys, c = ln(1/2048))
  PT = exp(E)   [k, q] layout
  outT = [Vp | 1]^T @ PT   (Vp = coef * Vh; ones column yields denom row)
  O = outT[0:64] / outT[64]    (per 128-q tile after a PE transpose, so the
                                reciprocal runs on 128 partitions)
  poutT[d, q] = [Wo_h0; Wo_h1]^T . [O_h0; O_h1]   (head-packed, K=128)

Schedule: chunk-granular input DMA on three queues so scores start ~12us in;
the scalar engine runs a pure exp stream (the critical resource); b2/norm/
finals operate on 512-column half-units so the tail pipelines; the PE is kept
dense (b1 of unit i+1 + b2 of unit i + norm transposes + finals as filler) to
hold the high DVFS p-state.
"""

import math
import os

os.environ.setdefault("MYCRO_LOCAL_CACHE", "1")

import numpy as np

import concourse.bass as bass
import concourse.tile as tile
from concourse import bacc
from concourse import mybir
from concourse.bass_utils import run_bass_kernel_spmd
from concourse.masks import make_identity

B = 2
L = 2048
DM = 512
H = 8
DH = 64
NCORES = 8
HPC = 2           # heads per core
DH2 = HPC * DH    # 128
NKT = L // 128    # 16 k tiles
QH = 1024         # q chunk per attention unit
NQH = L // QH     # 2
C_LN = -math.log(L)
NEG = -22.0       # large-negative for masked keys; e^(s-22) ~ 1e-10 is still
                  # nothing vs the denominator, and A16*x+B16 stays above the
                  # int16 wrap point in the fp16 exp trick (needs x > -32.5)

F32 = mybir.dt.float32
F16 = mybir.dt.float16
I32 = mybir.dt.int32

ATT_DT = F16      # exp output (PT), V'
MM_DT = F16       # matmul operand dtype
NP_MM = np.float16

# Schraudolph exp on DVE: int32(A*x + B) bit-read as fp32 approximates
# exp(x) (+-3% sawtooth); a second op converts the value to fp16.
SCHRAU_A = 12102203.161561485
SCHRAU_B = 1064866805.0
USE_DVE_EXP = True
DVE_KT = frozenset({2, 7, 12})

TRACE = False
LAST_RESULT = None

AUX_QM, AUX_ONES, AUX_KBMC, AUX_CLN = 0, 1, 2, 3


def build_nc(with_bias: bool):
    nc = bacc.Bacc(None, target_bir_lowering=False)

    xq_d = nc.declare_dram_parameter("xq", [4, 128, 4, 512], MM_DT, isOutput=False)
    xk_d = nc.declare_dram_parameter("xk", [4, 128, 4, 512], MM_DT, isOutput=False)
    xv_d = nc.declare_dram_parameter("xv", [4, 128, 4, 512], MM_DT, isOutput=False)
    wqs_d = nc.declare_dram_parameter("wqs", [128, 4, DH2], MM_DT, isOutput=False)
    wks_d = nc.declare_dram_parameter("wks", [128, 4, DH2], MM_DT, isOutput=False)
    wvs_d = nc.declare_dram_parameter("wvs", [128, 4, DH2], MM_DT, isOutput=False)
    wos_d = nc.declare_dram_parameter("wos", [DH2, DM], MM_DT, isOutput=False)
    aux_d = nc.declare_dram_parameter("aux", [4, L], MM_DT, isOutput=False)
    coef_d = nc.declare_dram_parameter("coef", [128, NKT], F32, isOutput=False)
    if with_bias:
        wbias_d = nc.declare_dram_parameter("wbias", [1, 4 * DH2], MM_DT, isOutput=False)
    pout_d = nc.declare_dram_parameter("poutT", [DM, L], MM_DT, isOutput=True)

    EXP = mybir.ActivationFunctionType.Exp

    with tile.TileContext(nc) as tc:
        with (
            tc.tile_pool(name="const", bufs=1) as const,
            tc.tile_pool(name="qek", bufs=1) as qek,
            tc.tile_pool(name="xin", bufs=1) as xin,
            tc.tile_pool(name="vtmp", bufs=1) as vtmp,
            tc.tile_pool(name="ptp", bufs=2) as ptp,
            tc.tile_pool(name="sbB", bufs=1) as sbB,
            tc.tile_pool(name="ps", bufs=1, space="PSUM") as ps,
        ):
            # persistent per-head extended operands
            QE = [qek.tile([66, L], MM_DT, name=f"QE{h}") for h in range(HPC)]
            KE = [qek.tile([66, L], MM_DT, name=f"KE{h}") for h in range(HPC)]
            Vp = [qek.tile([128, NKT, DH + 1], ATT_DT, name=f"Vp{h}") for h in range(HPC)]

            # ---- weight/aux DMAs on the scalar queue (idle until exp) ----
            wq_sb = const.tile([128, 4, DH2], MM_DT)
            nc.scalar.dma_start(out=wq_sb, in_=wqs_d[:, :, :])
            wk_sb = const.tile([128, 4, DH2], MM_DT)
            nc.scalar.dma_start(out=wk_sb, in_=wks_d[:, :, :])
            for h in range(HPC):
                nc.scalar.dma_start(out=QE[h][64:66, :], in_=aux_d[0:2, :])
                nc.scalar.dma_start(out=KE[h][64:66, :], in_=aux_d[2:4, :])

            # ---- x pieces: critical stream ALONE on sync (the DMA engine
            # pool is shared FIFO across queues — late pieces must not get
            # ahead of the k stream). Late pieces follow on the scalar queue;
            # the gpsimd queue is reserved for output drains.
            xq_sb = [xin.tile([128, 4, 512], MM_DT, tag="xq", bufs=4, name=f"xq{c}") for c in range(4)]
            xk_sb = [xin.tile([128, 4, 512], MM_DT, tag="xk", bufs=4, name=f"xk{c}") for c in range(4)]
            xv_sb = [xin.tile([128, 4, 512], MM_DT, tag="xv", bufs=4, name=f"xv{c}") for c in range(4)]
            for sb, dram, c in [
                (xq_sb[0], xq_d, 0), (xq_sb[1], xq_d, 1),
                (xk_sb[0], xk_d, 0), (xk_sb[1], xk_d, 1),
                (xk_sb[2], xk_d, 2), (xk_sb[3], xk_d, 3),
            ]:
                nc.sync.dma_start(out=sb, in_=dram[c])
            wv_sb = const.tile([128, 4, DH2], MM_DT)
            nc.scalar.dma_start(out=wv_sb, in_=wvs_d[:, :, :])
            coef_sb = const.tile([128, NKT], F32)
            nc.scalar.dma_start(out=coef_sb, in_=coef_d[:, :])
            for sb, dram, c in [
                (xq_sb[2], xq_d, 2), (xq_sb[3], xq_d, 3),
                (xv_sb[0], xv_d, 0), (xv_sb[1], xv_d, 1),
                (xv_sb[2], xv_d, 2), (xv_sb[3], xv_d, 3),
            ]:
                nc.scalar.dma_start(out=sb, in_=dram[c])
            wo_sb = const.tile([DH2, DM], MM_DT)
            nc.scalar.dma_start(out=wo_sb, in_=wos_d[:, :])
            if with_bias:
                wb_sb = const.tile([1, 4 * DH2], MM_DT)
                nc.scalar.dma_start(out=wb_sb, in_=wbias_d[:, :])
                qm_sb = const.tile([1, L], MM_DT)
                nc.scalar.dma_start(out=qm_sb, in_=aux_d[AUX_QM : AUX_QM + 1, :])
                ones_sb = const.tile([1, L], MM_DT)
                nc.scalar.dma_start(out=ones_sb, in_=aux_d[AUX_ONES : AUX_ONES + 1, :])

            # identities + Vp ones columns (off critical path)
            ident = const.tile([128, 128], MM_DT)
            make_identity(nc, ident)
            ident32 = const.tile([128, 128], F32)
            make_identity(nc, ident32)
            for h in range(HPC):
                nc.vector.memset(Vp[h][:, :, DH : DH + 1], 1.0)

            # warm the activation table (Exp) before the real stream begins
            warm = const.tile([1, 1], F32)
            nc.vector.memset(warm, 0.0)
            warm2 = const.tile([1, 1], F32)
            nc.scalar.activation(out=warm2, in_=warm, func=EXP)

            # ---- projections: one 512-col chunk at a time ----
            def proj_chunk(x_sb, w_sb, dst, ch, brow, brhs):
                psp = ps.tile([128, 512], F32, tag="sm", bufs=2, name="psp")
                for t in range(4):
                    nc.tensor.matmul(
                        psp,
                        lhsT=w_sb[:, t, :],
                        rhs=x_sb[ch][:, t, :],
                        start=(t == 0),
                        stop=(t == 3 and not with_bias),
                    )
                sl = slice(ch * 512, (ch + 1) * 512)
                if with_bias:
                    nc.tensor.matmul(
                        psp,
                        lhsT=wb_sb[0:1, brow * DH2 : (brow + 1) * DH2],
                        rhs=brhs[0:1, sl],
                        start=False,
                        stop=True,
                    )
                if dst is not None:
                    for h in range(HPC):
                        nc.vector.tensor_copy(
                            out=dst[h][0:DH, sl], in_=psp[h * DH : (h + 1) * DH, :]
                        )
                    return None
                return psp

            # ---- attention unit steps ----
            def b1_step(qh, h, pt, kt):
                st = ps.tile([128, QH], F32, tag="st", bufs=2, name="st")
                for c2 in range(2):
                    nc.tensor.matmul(
                        st[:, c2 * 512 : (c2 + 1) * 512],
                        lhsT=KE[h][0:66, kt * 128 : (kt + 1) * 128],
                        rhs=QE[h][0:66, qh * QH + c2 * 512 : qh * QH + (c2 + 1) * 512],
                        start=True,
                        stop=True,
                    )
                if USE_DVE_EXP and kt in DVE_KT:
                    t32 = sbB.tile([128, QH], I32, tag="dvt", bufs=2, name="t32")
                    nc.vector.tensor_scalar(
                        out=t32, in0=st,
                        scalar1=SCHRAU_A, scalar2=SCHRAU_B,
                        op0=mybir.AluOpType.mult, op1=mybir.AluOpType.add,
                    )
                    nc.vector.tensor_copy(out=pt[:, kt, :], in_=t32.bitcast(F32))
                else:
                    nc.scalar.activation(out=pt[:, kt, :], in_=st, func=EXP)

            def b1_steps(u, pt):
                qh, h = u
                for kt in range(NKT):
                    yield lambda kt=kt: b1_step(qh, h, pt, kt)

            def b2_half(u, pt, outph, c2):
                qh, h = u
                for kt in range(NKT):
                    def _s(kt=kt):
                        nc.tensor.matmul(
                            outph,
                            lhsT=Vp[h][:, kt, :],
                            rhs=pt[:, kt, c2 * 512 : (c2 + 1) * 512],
                            start=(kt == 0),
                            stop=(kt == NKT - 1),
                        )
                    yield _s

            def vproj_steps():
                VT_sb = vtmp.tile([128, L], MM_DT)
                for ch in range(4):
                    def _chunk(ch=ch):
                        psp = proj_chunk(xv_sb, wv_sb, None, ch, 2, None)
                        nc.vector.tensor_copy(
                            out=VT_sb[:, ch * 512 : (ch + 1) * 512], in_=psp
                        )
                    yield _chunk
                for kt in range(NKT):
                    def _tp(kt=kt):
                        tp = ps.tile([128, 128], MM_DT, tag="sm", bufs=2, name="tp")
                        nc.tensor.transpose(tp, VT_sb[:, kt * 128 : (kt + 1) * 128], ident)
                        for h in range(HPC):
                            nc.vector.tensor_scalar_mul(
                                out=Vp[h][:, kt, 0:DH],
                                in0=tp[:, h * DH : (h + 1) * DH],
                                scalar1=coef_sb[:, kt : kt + 1],
                            )
                    yield _tp

            # normalize one 512-col half of a unit
            def norm_half(u, outph, nrm, c2, use_scalar=False):
                qh, h = u
                ci = [0]
                def ccopy(out, in_):
                    # post-exp-stream: alternate scalar/vector to halve pacing
                    if use_scalar and ci[0] % 2 == 0:
                        nc.scalar.copy(out=out, in_=in_)
                    else:
                        nc.vector.tensor_copy(out=out, in_=in_)
                    ci[0] += 1
                outsb = sbB.tile([65, 512], F32, tag="outsb", bufs=2, name="outsb")
                tsb = sbB.tile([128, 4, 65], F32, tag="tsb", bufs=2, name="tsb")
                rcp = sbB.tile([128, 4], F32, tag="rcp", bufs=2, name="rcp")
                nrb = sbB.tile([128, 4, DH], ATT_DT, tag="nrb", bufs=2, name="nrb")
                def _cp():
                    ccopy(outsb, outph)
                yield _cp
                for j in range(4):
                    def _fwd(j=j):
                        tpf = ps.tile([128, 65], F32, tag="sm", bufs=2, name="tpf")
                        nc.tensor.transpose(
                            tpf, outsb[0:65, j * 128 : (j + 1) * 128], ident32[0:65, 0:65]
                        )
                        ccopy(tsb[:, j, :], tpf)
                    yield _fwd
                def _mid():
                    nc.vector.reciprocal(out=rcp, in_=tsb[:, :, DH : DH + 1])
                    nc.vector.tensor_tensor(
                        out=nrb,
                        in0=tsb[:, :, 0:DH],
                        in1=rcp.broadcast_to([128, 4, DH]),
                        op=mybir.AluOpType.mult,
                    )
                yield _mid
                for j in range(4):
                    def _bck(j=j):
                        tpb = ps.tile([DH, 128], ATT_DT, tag="sm", bufs=2, name="tpb")
                        nc.tensor.transpose(tpb, nrb[:, j, :], ident)
                        ccopy(
                            nrm[h * DH : (h + 1) * DH,
                                c2 * 512 + j * 128 : c2 * 512 + (j + 1) * 128],
                            tpb,
                        )
                    yield _bck

            def final_half(qh, nrm, c2, alt):
                # head-packed output projection for one 512-col half
                for dt4 in range(4):
                    def _f(dt4=dt4):
                        fin = ps.tile([128, 512], F32, tag="sm", bufs=2, name="fin")
                        nc.tensor.matmul(
                            fin,
                            lhsT=wo_sb[:, dt4 * 128 : (dt4 + 1) * 128],
                            rhs=nrm[:, c2 * 512 : (c2 + 1) * 512],
                            start=True,
                            stop=True,
                        )
                        fsb = sbB.tile([128, 512], MM_DT, tag="fsb", bufs=4, name="fsb")
                        if alt and (dt4 % 2 == 1):
                            nc.scalar.copy(out=fsb, in_=fin)
                        else:
                            nc.vector.tensor_copy(out=fsb, in_=fin)
                        dma_eng = nc.sync if (alt and dt4 % 2 == 0) else nc.gpsimd
                        dma_eng.dma_start(
                            out=pout_d[
                                dt4 * 128 : (dt4 + 1) * 128,
                                qh * QH + c2 * 512 : qh * QH + (c2 + 1) * 512,
                            ],
                            in_=fsb,
                        )
                    yield _f

            def chain(*gens):
                for g in gens:
                    yield from g

            def delayed(gen, n):
                for _ in range(n):
                    yield lambda: None
                yield from gen

            def pair2(gen):
                it = iter(gen)
                while True:
                    try:
                        a = next(it)
                    except StopIteration:
                        return
                    try:
                        b = next(it)
                    except StopIteration:
                        b = None
                    def step(a=a, b=b):
                        a()
                        if b is not None:
                            b()
                    yield step

            def interleave(*gens):
                gens = [iter(g) for g in gens if g is not None]
                while gens:
                    nxt = []
                    for g in gens:
                        try:
                            next(g)()
                        except StopIteration:
                            continue
                        nxt.append(g)
                    gens = nxt

            # ================= emission schedule =================
            units = [(0, 0), (0, 1), (1, 0), (1, 1)]
            pts = {}
            outps = {}
            nrms = {0: sbB.tile([128, QH], ATT_DT, tag="nrm0", name="nrm0"),
                    1: sbB.tile([128, QH], ATT_DT, tag="nrm1", name="nrm1")}

            def outp_half(name):
                return ps.tile([65, 512], F32, tag="outp", bufs=2, name=name)

            # minimum projections to start unit 0
            proj_chunk(xq_sb, wq_sb, QE, 0, 0, qm_sb if with_bias else None)
            proj_chunk(xq_sb, wq_sb, QE, 1, 0, qm_sb if with_bias else None)
            proj_chunk(xk_sb, wk_sb, KE, 0, 1, ones_sb if with_bias else None)

            def late_kproj():
                for ch in (1, 2, 3):
                    yield lambda ch=ch: proj_chunk(
                        xk_sb, wk_sb, KE, ch, 1, ones_sb if with_bias else None
                    )

            def late_qproj():
                for ch in (2, 3):
                    yield lambda ch=ch: proj_chunk(
                        xq_sb, wq_sb, QE, ch, 0, qm_sb if with_bias else None
                    )

            def b2_unit(i):
                outps[(i, 0)] = outp_half(f"outp{i}a")
                outps[(i, 1)] = outp_half(f"outp{i}b")
                return pair2(chain(
                    b2_half(units[i], pts[i], outps[(i, 0)], 0),
                    b2_half(units[i], pts[i], outps[(i, 1)], 1),
                ))

            # Phase A0: unit0 scores + remaining k projections only — steps
            # gated on late DMA pieces must not block this queue.
            pts[0] = ptp.tile([128, NKT, QH], ATT_DT, tag="pt", name="pt0")
            interleave(b1_steps(units[0], pts[0]), late_kproj())

            # Phase A1: b1(u1) + late q proj + v-projection + b2(u0);
            # norm(u0a) at the end. b2 must be emitted AFTER the vproj
            # transpose writing each Vp[kt] (emission order defines deps).
            pts[1] = ptp.tile([128, NKT, QH], ATT_DT, tag="pt", name="pt1")
            interleave(
                b1_steps(units[1], pts[1]),
                chain(late_qproj(), vproj_steps()),
                delayed(b2_unit(0), 16),
                delayed(norm_half(units[0], outps[(0, 0)], nrms[0], 0), 25),
            )

            # Phase A2: b1(u2) + b2(u1); norm(u0b), norm(u1a)
            pts[2] = ptp.tile([128, NKT, QH], ATT_DT, tag="pt", name="pt2")
            interleave(
                b1_steps(units[2], pts[2]),
                b2_unit(1),
                chain(
                    norm_half(units[0], outps[(0, 1)], nrms[0], 1),
                    norm_half(units[1], outps[(1, 0)], nrms[0], 0),
                ),
            )

            # Phase A3: b1(u3) + b2(u2); norm(u1b), finals(0,c0), norm(u2a), finals(0,c1)
            pts[3] = ptp.tile([128, NKT, QH], ATT_DT, tag="pt", name="pt3")
            interleave(
                b1_steps(units[3], pts[3]),
                b2_unit(2),
                chain(
                    norm_half(units[1], outps[(1, 1)], nrms[0], 1),
                    final_half(0, nrms[0], 0, alt=False),
                    norm_half(units[2], outps[(2, 0)], nrms[1], 0),
                    final_half(0, nrms[0], 1, alt=False),
                ),
            )

            # Tail: half-pipelined. Scalar engine is free (exp stream done).
            outps[(3, 0)] = outp_half("outp3a")
            outps[(3, 1)] = outp_half("outp3b")
            interleave(
                pair2(b2_half(units[3], pts[3], outps[(3, 0)], 0)),
                norm_half(units[2], outps[(2, 1)], nrms[1], 1, use_scalar=True),
            )
            interleave(
                pair2(b2_half(units[3], pts[3], outps[(3, 1)], 1)),
                norm_half(units[3], outps[(3, 0)], nrms[1], 0, use_scalar=True),
            )
            interleave(
                norm_half(units[3], outps[(3, 1)], nrms[1], 1, use_scalar=True),
                final_half(1, nrms[1], 0, alt=True),
            )
            for s in final_half(1, nrms[1], 1, alt=True):
                s()

    nc.compile()
    return nc


_CACHE = {}


def _get_nc(with_bias: bool):
    key = ("nc", with_bias, USE_DVE_EXP, tuple(sorted(DVE_KT)))
    if key not in _CACHE:
        _CACHE[key] = build_nc(with_bias)
    return _CACHE[key]


def kernel(q, k, v, text_mask, audio_mask, n_head, wq, bq, wk, bk, wv, bv, wo, bo):
    global LAST_RESULT
    q = np.asarray(q, np.float32)
    k = np.asarray(k, np.float32)
    v = np.asarray(v, np.float32)
    text_mask = np.asarray(text_mask, np.float32)
    audio_mask = np.asarray(audio_mask, np.float32)
    wq = np.asarray(wq, np.float32)
    wk = np.asarray(wk, np.float32)
    wv = np.asarray(wv, np.float32)
    wo = np.asarray(wo, np.float32)
    bq = np.asarray(bq, np.float32)
    bk = np.asarray(bk, np.float32)
    bv = np.asarray(bv, np.float32)
    bo = np.asarray(bo, np.float32)
    assert int(n_head) == H

    with_bias = bool(np.any(bq) or np.any(bk) or np.any(bv))

    pad = np.concatenate([text_mask, audio_mask], axis=1)  # [B, L]
    qm = (pad != 0).astype(np.float32)
    tl = text_mask.sum(1)
    al = audio_mask.sum(1)
    tot = tl + al
    coef = np.concatenate(
        [
            text_mask * (tot / (2.0 * tl))[:, None],
            audio_mask * (tot / (2.0 * al))[:, None],
        ],
        axis=1,
    ).astype(np.float32)
    kbmc = (NEG * (1.0 - qm) - C_LN).astype(np.float32)
    ones_row = np.ones((L,), np.float32)
    cln_row = np.full((L,), C_LN, np.float32)

    def cc(a):
        return np.ascontiguousarray(a, dtype=NP_MM)

    def pieces(xT):
        # [DM, L] -> [ch, p, t, m] so each 512-col piece is contiguous
        return np.ascontiguousarray(
            xT.reshape(4, 128, 4, 512).transpose(2, 1, 0, 3), dtype=NP_MM
        )

    def wpack(wT):
        # [DM, DH2] -> [p, t, DH2]
        return np.ascontiguousarray(
            wT.reshape(4, 128, DH2).transpose(1, 0, 2), dtype=NP_MM
        )

    in_maps = []
    for core in range(NCORES):
        b, hp = divmod(core, NCORES // B)
        cols = slice(hp * DH2, (hp + 1) * DH2)
        m = {
            "xq": pieces((q[b] * qm[b][:, None]).T.astype(np.float32)),
            "xk": pieces(k[b].T.astype(np.float32)),
            "xv": pieces(v[b].T.astype(np.float32)),
            "wqs": wpack(wq.T[:, cols]),
            "wks": wpack(wk.T[:, cols] / 8.0),
            "wvs": wpack(wv.T[:, cols]),
            "wos": cc(wo.T[cols, :]),
            "aux": cc(np.stack([qm[b], ones_row, kbmc[b], cln_row])),
            "coef": np.ascontiguousarray(
                coef[b].reshape(NKT, 128).T, dtype=np.float32
            ),
        }
        if with_bias:
            m["wbias"] = cc(
                np.concatenate(
                    [bq[cols], bk[cols] / 8.0, bv[cols], np.zeros(DH2, np.float32)]
                )
            ).reshape(1, 4 * DH2)
        in_maps.append(m)

    res = run_bass_kernel_spmd(
        _get_nc(with_bias), in_maps, core_ids=list(range(NCORES)), trace=TRACE
    )
    LAST_RESULT = res

    out = np.zeros((B, L, DM), np.float32)
    npc = NCORES // B
    for b in range(B):
        acc = res.results[b * npc]["poutT"].astype(np.float32)
        for hp in range(1, npc):
            acc = acc + res.results[b * npc + hp]["poutT"].astype(np.float32)
        out[b] = acc.T + bo[None, :]
    return out


# revision 34
# speedup vs baseline: 1.0048x; 1.0048x over previous
"""Trainium2 Bass kernel for CustomMultiHeadAttention (B=2, L=2048, D=512, H=8).

Sharding: 8 cores = 2 batches x 4 head-pairs. Each core computes, for its
batch b and its 2 heads, the partial output (O_h @ Wo_h summed over its
heads), transposed: poutT [512, 2048]. Host sums the 4 partials per batch,
transposes, and adds bo.

Device-side math per core (all masking folded into matmul contractions):
  Qh = (q[b]*qm) @ WqT_cols + qm*bq_cols          (masked q rows -> exactly 0)
  Kh = k[b] @ WkT_cols/8 + bk_cols/8
  E[k,q] = Kh.Qh + (kb[k]-c)*qm[q] + c            via 2 extra contraction rows
           (kb = -100 for masked keys, c = ln(1/2048))
  PT = exp(E)   [k, q] layout
  outT = [Vp | 1]^T @ PT   (Vp = coef * Vh; ones column yields denom row)
  O = outT[0:64] / outT[64]    (per 128-q tile after a PE transpose, so the
                                reciprocal runs on 128 partitions)
  poutT[d, q] = [Wo_h0; Wo_h1]^T . [O_h0; O_h1]   (head-packed, K=128)

Schedule: chunk-granular input DMA on three queues so scores start ~12us in;
the scalar engine runs a pure exp stream (the critical resource); b2/norm/
finals operate on 512-column half-units so the tail pipelines; the PE is kept
dense (b1 of unit i+1 + b2 of unit i + norm transposes + finals as filler) to
hold the high DVFS p-state.
"""

import math
import os

os.environ.setdefault("MYCRO_LOCAL_CACHE", "1")

import numpy as np

import concourse.bass as bass
import concourse.tile as tile
from concourse import bacc
from concourse import mybir
from concourse.bass_utils import run_bass_kernel_spmd
from concourse.masks import make_identity

B = 2
L = 2048
DM = 512
H = 8
DH = 64
NCORES = 8
HPC = 2           # heads per core
DH2 = HPC * DH    # 128
NKT = L // 128    # 16 k tiles
QH = 1024         # q chunk per attention unit
NQH = L // QH     # 2
C_LN = -math.log(L)
NEG = -22.0       # large-negative for masked keys; e^(s-22) ~ 1e-10 is still
                  # nothing vs the denominator, and A16*x+B16 stays above the
                  # int16 wrap point in the fp16 exp trick (needs x > -32.5)

F32 = mybir.dt.float32
F16 = mybir.dt.float16
I32 = mybir.dt.int32

ATT_DT = F16      # exp output (PT), V'
MM_DT = F16       # matmul operand dtype
NP_MM = np.float16

# Schraudolph exp on DVE: int32(A*x + B) bit-read as fp32 approximates
# exp(x) (+-3% sawtooth); a second op converts the value to fp16.
SCHRAU_A = 12102203.161561485
SCHRAU_B = 1064866805.0
USE_DVE_EXP = True
DVE_KT = frozenset({2, 7, 12})

TRACE = False
LAST_RESULT = None

AUX_QM, AUX_ONES, AUX_KBMC, AUX_CLN = 0, 1, 2, 3


def build_nc(with_bias: bool):
    nc = bacc.Bacc(None, target_bir_lowering=False)

    xq_d = nc.declare_dram_parameter("xq", [4, 128, 4, 512], MM_DT, isOutput=False)
    xk_d = nc.declare_dram_parameter("xk", [4, 128, 4, 512], MM_DT, isOutput=False)
    xv_d = nc.declare_dram_parameter("xv", [4, 128, 4, 512], MM_DT, isOutput=False)
    wqs_d = nc.declare_dram_parameter("wqs", [128, 4, DH2], MM_DT, isOutput=False)
    wks_d = nc.declare_dram_parameter("wks", [128, 4, DH2], MM_DT, isOutput=False)
    wvs_d = nc.declare_dram_parameter("wvs", [128, 4, DH2], MM_DT, isOutput=False)
    wos_d = nc.declare_dram_parameter("wos", [DH2, DM], MM_DT, isOutput=False)
    aux_d = nc.declare_dram_parameter("aux", [4, L], MM_DT, isOutput=False)
    coef_d = nc.declare_dram_parameter("coef", [128, NKT], F32, isOutput=False)
    if with_bias:
        wbias_d = nc.declare_dram_parameter("wbias", [1, 4 * DH2], MM_DT, isOutput=False)
    pout_d = nc.declare_dram_parameter("poutT", [DM, L], MM_DT, isOutput=True)

    EXP = mybir.ActivationFunctionType.Exp

    with tile.TileContext(nc) as tc:
        with (
            tc.tile_pool(name="const", bufs=1) as const,
            tc.tile_pool(name="qek", bufs=1) as qek,
            tc.tile_pool(name="xin", bufs=1) as xin,
            tc.tile_pool(name="vtmp", bufs=1) as vtmp,
            tc.tile_pool(name="ptp", bufs=2) as ptp,
            tc.tile_pool(name="sbB", bufs=1) as sbB,
            tc.tile_pool(name="ps", bufs=1, space="PSUM") as ps,
        ):
            # persistent per-head extended operands
            QE = [qek.tile([66, L], MM_DT, name=f"QE{h}") for h in range(HPC)]
            KE = [qek.tile([66, L], MM_DT, name=f"KE{h}") for h in range(HPC)]
            Vp = [qek.tile([128, NKT, DH + 1], ATT_DT, name=f"Vp{h}") for h in range(HPC)]

            # ---- weight/aux DMAs on the scalar queue (idle until exp) ----
            wq_sb = const.tile([128, 4, DH2], MM_DT)
            nc.scalar.dma_start(out=wq_sb, in_=wqs_d[:, :, :])
            wk_sb = const.tile([128, 4, DH2], MM_DT)
            nc.scalar.dma_start(out=wk_sb, in_=wks_d[:, :, :])
            for h in range(HPC):
                nc.scalar.dma_start(out=QE[h][64:66, :], in_=aux_d[0:2, :])
                nc.scalar.dma_start(out=KE[h][64:66, :], in_=aux_d[2:4, :])

            # ---- x pieces: critical stream ALONE on sync (the DMA engine
            # pool is shared FIFO across queues — late pieces must not get
            # ahead of the k stream). Late pieces follow on the scalar queue;
            # the gpsimd queue is reserved for output drains.
            xq_sb = [xin.tile([128, 4, 512], MM_DT, tag="xq", bufs=4, name=f"xq{c}") for c in range(4)]
            xk_sb = [xin.tile([128, 4, 512], MM_DT, tag="xk", bufs=4, name=f"xk{c}") for c in range(4)]
            xv_sb = [xin.tile([128, 4, 512], MM_DT, tag="xv", bufs=4, name=f"xv{c}") for c in range(4)]
            for sb, dram, c in [
                (xq_sb[0], xq_d, 0), (xq_sb[1], xq_d, 1),
                (xk_sb[0], xk_d, 0), (xk_sb[1], xk_d, 1),
                (xk_sb[2], xk_d, 2), (xk_sb[3], xk_d, 3),
            ]:
                nc.sync.dma_start(out=sb, in_=dram[c])
            wv_sb = const.tile([128, 4, DH2], MM_DT)
            nc.scalar.dma_start(out=wv_sb, in_=wvs_d[:, :, :])
            coef_sb = const.tile([128, NKT], F32)
            nc.scalar.dma_start(out=coef_sb, in_=coef_d[:, :])
            for sb, dram, c in [
                (xq_sb[2], xq_d, 2), (xq_sb[3], xq_d, 3),
                (xv_sb[0], xv_d, 0), (xv_sb[1], xv_d, 1),
                (xv_sb[2], xv_d, 2), (xv_sb[3], xv_d, 3),
            ]:
                nc.scalar.dma_start(out=sb, in_=dram[c])
            wo_sb = const.tile([DH2, DM], MM_DT)
            nc.scalar.dma_start(out=wo_sb, in_=wos_d[:, :])
            if with_bias:
                wb_sb = const.tile([1, 4 * DH2], MM_DT)
                nc.scalar.dma_start(out=wb_sb, in_=wbias_d[:, :])
                qm_sb = const.tile([1, L], MM_DT)
                nc.scalar.dma_start(out=qm_sb, in_=aux_d[AUX_QM : AUX_QM + 1, :])
                ones_sb = const.tile([1, L], MM_DT)
                nc.scalar.dma_start(out=ones_sb, in_=aux_d[AUX_ONES : AUX_ONES + 1, :])

            # identities + Vp ones columns (off critical path)
            ident = const.tile([128, 128], MM_DT)
            make_identity(nc, ident)
            ident32 = const.tile([128, 128], F32)
            make_identity(nc, ident32)
            for h in range(HPC):
                nc.vector.memset(Vp[h][:, :, DH : DH + 1], 1.0)

            # warm the activation table (Exp) before the real stream begins
            warm = const.tile([1, 1], F32)
            nc.vector.memset(warm, 0.0)
            warm2 = const.tile([1, 1], F32)
            nc.scalar.activation(out=warm2, in_=warm, func=EXP)

            # ---- projections: one 512-col chunk at a time ----
            def proj_chunk(x_sb, w_sb, dst, ch, brow, brhs):
                psp = ps.tile([128, 512], F32, tag="sm", bufs=2, name="psp")
                for t in range(4):
                    nc.tensor.matmul(
                        psp,
                        lhsT=w_sb[:, t, :],
                        rhs=x_sb[ch][:, t, :],
                        start=(t == 0),
                        stop=(t == 3 and not with_bias),
                    )
                sl = slice(ch * 512, (ch + 1) * 512)
                if with_bias:
                    nc.tensor.matmul(
                        psp,
                        lhsT=wb_sb[0:1, brow * DH2 : (brow + 1) * DH2],
                        rhs=brhs[0:1, sl],
                        start=False,
                        stop=True,
                    )
                if dst is not None:
                    for h in range(HPC):
                        nc.vector.tensor_copy(
                            out=dst[h][0:DH, sl], in_=psp[h * DH : (h + 1) * DH, :]
                        )
                    return None
                return psp

            # ---- attention unit steps ----
            def b1_step(qh, h, pt, kt):
                st = ps.tile([128, QH], F32, tag="st", bufs=2, name="st")
                for c2 in range(2):
                    nc.tensor.matmul(
                        st[:, c2 * 512 : (c2 + 1) * 512],
                        lhsT=KE[h][0:66, kt * 128 : (kt + 1) * 128],
                        rhs=QE[h][0:66, qh * QH + c2 * 512 : qh * QH + (c2 + 1) * 512],
                        start=True,
                        stop=True,
                    )
                if USE_DVE_EXP and kt in DVE_KT:
                    t32 = sbB.tile([128, QH], I32, tag="dvt", bufs=2, name="t32")
                    nc.vector.tensor_scalar(
                        out=t32, in0=st,
                        scalar1=SCHRAU_A, scalar2=SCHRAU_B,
                        op0=mybir.AluOpType.mult, op1=mybir.AluOpType.add,
                    )
                    nc.vector.tensor_copy(out=pt[:, kt, :], in_=t32.bitcast(F32))
                else:
                    nc.scalar.activation(out=pt[:, kt, :], in_=st, func=EXP)

            def b1_steps(u, pt):
                qh, h = u
                for kt in range(NKT):
                    yield lambda kt=kt: b1_step(qh, h, pt, kt)

            def b2_half(u, pt, outph, c2):
                qh, h = u
                for kt in range(NKT):
                    def _s(kt=kt):
                        nc.tensor.matmul(
                            outph,
                            lhsT=Vp[h][:, kt, :],
                            rhs=pt[:, kt, c2 * 512 : (c2 + 1) * 512],
                            start=(kt == 0),
                            stop=(kt == NKT - 1),
                        )
                    yield _s

            def vproj_steps():
                VT_sb = vtmp.tile([128, L], MM_DT)
                for ch in range(4):
                    def _chunk(ch=ch):
                        psp = proj_chunk(xv_sb, wv_sb, None, ch, 2, None)
                        nc.vector.tensor_copy(
                            out=VT_sb[:, ch * 512 : (ch + 1) * 512], in_=psp
                        )
                    yield _chunk
                for kt in range(NKT):
                    def _tp(kt=kt):
                        tp = ps.tile([128, 128], MM_DT, tag="sm", bufs=2, name="tp")
                        nc.tensor.transpose(tp, VT_sb[:, kt * 128 : (kt + 1) * 128], ident)
                        for h in range(HPC):
                            nc.vector.tensor_scalar_mul(
                                out=Vp[h][:, kt, 0:DH],
                                in0=tp[:, h * DH : (h + 1) * DH],
                                scalar1=coef_sb[:, kt : kt + 1],
                            )
                    yield _tp

            # normalize one 512-col half of a unit
            def norm_half(u, outph, nrm, c2, use_scalar=False):
                qh, h = u
                ci = [0]
                def ccopy(out, in_):
                    # post-exp-stream: alternate scalar/vector to halve pacing
                    if use_scalar and ci[0] % 2 == 0:
                        nc.scalar.copy(out=out, in_=in_)
                    else:
                        nc.vector.tensor_copy(out=out, in_=in_)
                    ci[0] += 1
                outsb = sbB.tile([65, 512], F32, tag="outsb", bufs=2, name="outsb")
                tsb = sbB.tile([128, 4, 65], F32, tag="tsb", bufs=2, name="tsb")
                rcp = sbB.tile([128, 4], F32, tag="rcp", bufs=2, name="rcp")
                nrb = sbB.tile([128, 4, DH], ATT_DT, tag="nrb", bufs=2, name="nrb")
                def _cp():
                    ccopy(outsb, outph)
                yield _cp
                for j in range(4):
                    def _fwd(j=j):
                        tpf = ps.tile([128, 65], F32, tag="sm", bufs=2, name="tpf")
                        nc.tensor.transpose(
                            tpf, outsb[0:65, j * 128 : (j + 1) * 128], ident32[0:65, 0:65]
                        )
                        ccopy(tsb[:, j, :], tpf)
                    yield _fwd
                def _mid():
                    nc.vector.reciprocal(out=rcp, in_=tsb[:, :, DH : DH + 1])
                    nc.vector.tensor_tensor(
                        out=nrb,
                        in0=tsb[:, :, 0:DH],
                        in1=rcp.broadcast_to([128, 4, DH]),
                        op=mybir.AluOpType.mult,
                    )
                yield _mid
                for j in range(4):
                    def _bck(j=j):
                        tpb = ps.tile([DH, 128], ATT_DT, tag="sm", bufs=2, name="tpb")
                        nc.tensor.transpose(tpb, nrb[:, j, :], ident)
                        ccopy(
                            nrm[h * DH : (h + 1) * DH,
                                c2 * 512 + j * 128 : c2 * 512 + (j + 1) * 128],
                            tpb,
                        )
                    yield _bck

            def final_half(qh, nrm, c2, alt):
                # head-packed output projection for one 512-col half
                for dt4 in range(4):
                    def _f(dt4=dt4):
                        fin = ps.tile([128, 512], F32, tag="sm", bufs=2, name="fin")
                        nc.tensor.matmul(
                            fin,
                            lhsT=wo_sb[:, dt4 * 128 : (dt4 + 1) * 128],
                            rhs=nrm[:, c2 * 512 : (c2 + 1) * 512],
                            start=True,
                            stop=True,
                        )
                        fsb = sbB.tile([128, 512], MM_DT, tag="fsb", bufs=4, name="fsb")
                        if alt and (dt4 % 2 == 1):
                            nc.scalar.copy(out=fsb, in_=fin)
                        else:
                            nc.vector.tensor_copy(out=fsb, in_=fin)
                        dma_eng = nc.sync if (alt and dt4 % 2 == 0) else nc.gpsimd
                        dma_eng.dma_start(
                            out=pout_d[
                                dt4 * 128 : (dt4 + 1) * 128,
                                qh * QH + c2 * 512 : qh * QH + (c2 + 1) * 512,
                            ],
                            in_=fsb,
                        )
                    yield _f

            def chain(*gens):
                for g in gens:
                    yield from g

            def delayed(gen, n):
                for _ in range(n):
                    yield lambda: None
                yield from gen

            def pair2(gen):
                it = iter(gen)
                while True:
                    try:
                        a = next(it)
                    except StopIteration:
                        return
                    try:
                        b = next(it)
                    except StopIteration:
                        b = None
                    def step(a=a, b=b):
                        a()
                        if b is not None:
                            b()
                    yield step

            def interleave(*gens):
                gens = [iter(g) for g in gens if g is not None]
                while gens:
                    nxt = []
                    for g in gens:
                        try:
                            next(g)()
                        except StopIteration:
                            continue
                        nxt.append(g)
                    gens = nxt

            # ================= emission schedule =================
            units = [(0, 0), (0, 1), (1, 0), (1, 1)]
            pts = {}
            outps = {}
            nrms = {0: sbB.tile([128, QH], ATT_DT, tag="nrm0", name="nrm0"),
                    1: sbB.tile([128, QH], ATT_DT, tag="nrm1", name="nrm1")}

            def outp_half(name):
                return ps.tile([65, 512], F32, tag="outp", bufs=2, name=name)

            # minimum projections to start unit 0
            proj_chunk(xq_sb, wq_sb, QE, 0, 0, qm_sb if with_bias else None)
            proj_chunk(xq_sb, wq_sb, QE, 1, 0, qm_sb if with_bias else None)
            proj_chunk(xk_sb, wk_sb, KE, 0, 1, ones_sb if with_bias else None)

            def late_kproj():
                for ch in (1, 2, 3):
                    yield lambda ch=ch: proj_chunk(
                        xk_sb, wk_sb, KE, ch, 1, ones_sb if with_bias else None
                    )

            def late_qproj():
                for ch in (2, 3):
                    yield lambda ch=ch: proj_chunk(
                        xq_sb, wq_sb, QE, ch, 0, qm_sb if with_bias else None
                    )

            def b2_unit(i):
                outps[(i, 0)] = outp_half(f"outp{i}a")
                outps[(i, 1)] = outp_half(f"outp{i}b")
                return pair2(chain(
                    b2_half(units[i], pts[i], outps[(i, 0)], 0),
                    b2_half(units[i], pts[i], outps[(i, 1)], 1),
                ))

            # Phase A0: unit0 scores + remaining k projections only — steps
            # gated on late DMA pieces must not block this queue.
            pts[0] = ptp.tile([128, NKT, QH], ATT_DT, tag="pt", name="pt0")
            interleave(b1_steps(units[0], pts[0]), late_kproj())

            # Phase A1: b1(u1) + late q proj + v-projection + b2(u0);
            # norm(u0a) at the end. b2 must be emitted AFTER the vproj
            # transpose writing each Vp[kt] (emission order defines deps).
            pts[1] = ptp.tile([128, NKT, QH], ATT_DT, tag="pt", name="pt1")
            interleave(
                b1_steps(units[1], pts[1]),
                chain(late_qproj(), vproj_steps()),
                delayed(b2_unit(0), 16),
                delayed(norm_half(units[0], outps[(0, 0)], nrms[0], 0), 25),
            )

            # Phase A2: b1(u2) + b2(u1); norm(u0b), norm(u1a)
            pts[2] = ptp.tile([128, NKT, QH], ATT_DT, tag="pt", name="pt2")
            interleave(
                b1_steps(units[2], pts[2]),
                b2_unit(1),
                chain(
                    norm_half(units[0], outps[(0, 1)], nrms[0], 1),
                    norm_half(units[1], outps[(1, 0)], nrms[0], 0),
                ),
            )

            # Phase A3: b1(u3) + b2(u2); norm(u1b), finals(0,c0), norm(u2a), finals(0,c1)
            pts[3] = ptp.tile([128, NKT, QH], ATT_DT, tag="pt", name="pt3")
            interleave(
                b1_steps(units[3], pts[3]),
                b2_unit(2),
                chain(
                    norm_half(units[1], outps[(1, 1)], nrms[0], 1),
                    final_half(0, nrms[0], 0, alt=False),
                    norm_half(units[2], outps[(2, 0)], nrms[1], 0),
                    final_half(0, nrms[0], 1, alt=False),
                ),
            )

            # Tail: half-pipelined. Scalar engine is free (exp stream done).
            outps[(3, 0)] = outp_half("outp3a")
            outps[(3, 1)] = outp_half("outp3b")
            interleave(
                pair2(b2_half(units[3], pts[3], outps[(3, 0)], 0)),
                norm_half(units[2], outps[(2, 1)], nrms[1], 1, use_scalar=True),
            )
            interleave(
                pair2(b2_half(units[3], pts[3], outps[(3, 1)], 1)),
                norm_half(units[3], outps[(3, 0)], nrms[1], 0, use_scalar=True),
            )
            interleave(
                norm_half(units[3], outps[(3, 1)], nrms[1], 1, use_scalar=True),
                final_half(1, nrms[1], 0, alt=True),
            )
            for s in final_half(1, nrms[1], 1, alt=True):
                s()

    nc.compile()
    return nc


_CACHE = {}


def _get_nc(with_bias: bool):
    key = ("nc", with_bias, USE_DVE_EXP, tuple(sorted(DVE_KT)))
    if key not in _CACHE:
        _CACHE[key] = build_nc(with_bias)
    return _CACHE[key]


def kernel(q, k, v, text_mask, audio_mask, n_head, wq, bq, wk, bk, wv, bv, wo, bo):
    global LAST_RESULT
    q = np.asarray(q, np.float32)
    k = np.asarray(k, np.float32)
    v = np.asarray(v, np.float32)
    text_mask = np.asarray(text_mask, np.float32)
    audio_mask = np.asarray(audio_mask, np.float32)
    wq = np.asarray(wq, np.float32)
    wk = np.asarray(wk, np.float32)
    wv = np.asarray(wv, np.float32)
    wo = np.asarray(wo, np.float32)
    bq = np.asarray(bq, np.float32)
    bk = np.asarray(bk, np.float32)
    bv = np.asarray(bv, np.float32)
    bo = np.asarray(bo, np.float32)
    assert int(n_head) == H

    with_bias = bool(np.any(bq) or np.any(bk) or np.any(bv))

    pad = np.concatenate([text_mask, audio_mask], axis=1)  # [B, L]
    qm = (pad != 0).astype(np.float32)
    tl = text_mask.sum(1)
    al = audio_mask.sum(1)
    tot = tl + al
    coef = np.concatenate(
        [
            text_mask * (tot / (2.0 * tl))[:, None],
            audio_mask * (tot / (2.0 * al))[:, None],
        ],
        axis=1,
    ).astype(np.float32)
    kbmc = (NEG * (1.0 - qm) - C_LN).astype(np.float32)
    ones_row = np.ones((L,), np.float32)
    cln_row = np.full((L,), C_LN, np.float32)

    def cc(a):
        return np.ascontiguousarray(a, dtype=NP_MM)

    def pieces(xT):
        # [DM, L] -> [ch, p, t, m] so each 512-col piece is contiguous
        return np.ascontiguousarray(
            xT.reshape(4, 128, 4, 512).transpose(2, 1, 0, 3), dtype=NP_MM
        )

    def wpack(wT):
        # [DM, DH2] -> [p, t, DH2]
        return np.ascontiguousarray(
            wT.reshape(4, 128, DH2).transpose(1, 0, 2), dtype=NP_MM
        )

    in_maps = []
    for core in range(NCORES):
        b, hp = divmod(core, NCORES // B)
        cols = slice(hp * DH2, (hp + 1) * DH2)
        m = {
            "xq": pieces((q[b] * qm[b][:, None]).T.astype(np.float32)),
            "xk": pieces(k[b].T.astype(np.float32)),
            "xv": pieces(v[b].T.astype(np.float32)),
            "wqs": wpack(wq.T[:, cols]),
            "wks": wpack(wk.T[:, cols] / 8.0),
            "wvs": wpack(wv.T[:, cols]),
            "wos": cc(wo.T[cols, :]),
            "aux": cc(np.stack([qm[b], ones_row, kbmc[b], cln_row])),
            "coef": np.ascontiguousarray(
                coef[b].reshape(NKT, 128).T, dtype=np.float32
            ),
        }
        if with_bias:
            m["wbias"] = cc(
                np.concatenate(
                    [bq[cols], bk[cols] / 8.0, bv[cols], np.zeros(DH2, np.float32)]
                )
            ).reshape(1, 4 * DH2)
        in_maps.append(m)

    res = run_bass_kernel_spmd(
        _get_nc(with_bias), in_maps, core_ids=list(range(NCORES)), trace=TRACE
    )
    LAST_RESULT = res

    out = np.zeros((B, L, DM), np.float32)
    npc = NCORES // B
    for b in range(B):
        acc = res.results[b * npc]["poutT"].astype(np.float32)
        for hp in range(1, npc):
            acc = acc + res.results[b * npc + hp]["poutT"].astype(np.float32)
        out[b] = acc.T + bo[None, :]
    return out
